# revision 27
# baseline (speedup 1.0000x reference)
"""FDGNN (gnn_message_passing) Trainium2 kernel, 8-core SPMD.

Strategy (v2):
- Only 3 of the reference's 6 convs feed the output:
    s1 = conv_i2s(xi0); i2 = conv_s2i(s1); s3 = conv_i2s(i2); out = tanh(s3@wo+bo)
- mlp_m commutes with the per-edge gather: mlp_m(x[src]) == mlp_m(x)[src], so
  the message MLP runs per *node* (12.5K rows/core), not per *edge*.
- Destination nodes are sharded across the 8 cores. Each conv:
    1. y = mlp_m(x)  computed into SBUF (ybuf, partition-major, rows duplicated
       to 256B for the dma_gather 256B-element granule)
    2. y written to DRAM in 4 quarter slabs (one contiguous DMA each), each
       followed by a quarter-AllGather into that chunk's shared table. Chunk
       q's gathers start as soon as quarter q lands -> collective pipelined
       behind compute.
    3. chunk-major segment-sum: for each chunk, dma_gather edge source rows
       (int16 indices < 25600), build one-hot S tiles on DVE via
       tensor_scalar(iota_bf16, ==, drel), accumulate
       psum[64,128] += gb[:, t, 0:64].T @ S on the PE, then copy/add into
       aggT (fp32, feature-major).
    4. x' = mlp_u(aggT)  (bf16 matmuls, fp32 psum)
- Final: out = tanh(x @ wo + bo) per 128-row tile, staged in SBUF, one
  contiguous DMA out (partition-major; host unpermutes rows).
"""

import os as _os

import numpy as np
import ml_dtypes

NCORES = 8
NNODE = 100000  # both NS and NI
PERCORE = NNODE // NCORES  # 12500
NW = 98  # windows per core (98*128 = 12544)
PADPER = NW * 128  # 12544 padded rows per core
D = 64
HM = 32
HU = 16

NCHUNK = 4
QW = [25, 25, 24, 24]  # windows per quarter
QJ0 = [0, 25, 50, 74]  # first window of each quarter
RQ = [128 * w for w in QW]  # y rows per core per quarter
CHUNK_ROWS = [NCORES * r for r in RQ]  # table rows per chunk (<= 25600, int16-safe)

GT = int(_os.environ.get("KGT", "16"))  # tiles per dma_gather call
NQUEUES = int(_os.environ.get("KNQ", "4"))  # SWDGE queues (1-4)
SCRATCH = int(_os.environ.get("KSCRATCH", "32768"))

TRACE = False  # set by test harness to capture an NTFF profile
LAST_RESULT = None  # BassKernelResults of the most recent run


# ---------------------------------------------------------------- host prep

def _prep_relation(src, dst):
    """Route edges (dst-sharded) into per-core, per-chunk gather streams.

    Chunk q of the table = quarter-AllGather of y source windows
    [QJ0[q], QJ0[q]+QW[q]); within a core's slab rows are (r, j) =
    (src%128, src//128 - QJ0[q]) in partition-major order r*QW[q]+(j-j0).
    """
    E = src.shape[0]
    src = src.astype(np.int64)
    dst = dst.astype(np.int64)

    p = dst // PERCORE  # dst core
    dl = dst - p * PERCORE
    w = dl >> 7  # dst window
    drel = dl & 127

    ps_ = src // PERCORE  # src core
    sl = src - ps_ * PERCORE
    r = sl & 127
    j = sl >> 7  # src window
    q = (j >= 25).astype(np.int64) + (j >= 50) + (j >= 74)
    qw = np.array(QW, np.int64)[q]
    qj0 = np.array(QJ0, np.int64)[q]
    lidx = ps_ * (128 * qw) + r * qw + (j - qj0)  # row within chunk q

    key = (p * NCHUNK + q) * NW + w
    counts = np.bincount(key, minlength=NCORES * NCHUNK * NW).reshape(
        NCORES, NCHUNK, NW
    )
    ntiles_qw = -(-counts.max(axis=0) // 128)  # [NCHUNK, NW]
    ntiles_qw[0] = np.maximum(ntiles_qw[0], 1)  # chunk 0 initializes every window
    N_qw = ntiles_qw * 128
    base_w = np.zeros((NCHUNK, NW + 1), np.int64)
    base_w[:, 1:] = np.cumsum(N_qw, axis=1)
    T_q = (base_w[:, -1] // 128).astype(np.int64)  # tiles per chunk stream

    # rank of each edge within its (p, q, w) cell
    order = np.argsort(key, kind="stable")
    kk = key[order]
    grp_first = np.r_[True, kk[1:] != kk[:-1]]
    first_pos = np.flatnonzero(grp_first)
    starts = np.repeat(first_pos, np.diff(np.r_[first_pos, E]))
    rank = np.arange(E) - starts
    inv = np.empty(E, np.int64)
    inv[order] = rank
    slot = base_w[q, w] + inv  # slot within (core, chunk) stream

    idx_streams = []  # [core][chunk] -> int16 [128, T_q*8] packed
    drel_streams = []  # [core][chunk] -> bf16 [128, T_q]
    for pp in range(NCORES):
        rows_i = []
        rows_d = []
        pm = p == pp
        for qq in range(NCHUNK):
            n = int(T_q[qq]) * 128
            ar = np.arange(n)
            # pad slots gather scattered (valid, finite) rows; S row is 0
            idx_flat = (ar * 97) % CHUNK_ROWS[qq]
            drel_flat = np.full(n, -1.0, np.float32)
            m = pm & (q == qq)
            idx_flat[slot[m]] = lidx[m]
            drel_flat[slot[m]] = drel[m]
            assert idx_flat.max() < CHUNK_ROWS[qq] and idx_flat.min() >= 0
            idx16 = idx_flat.astype(np.int16)
            packed = np.tile(idx16.reshape(n // 16, 16).T, (8, 1))  # [128, n/16]
            rows_i.append(np.ascontiguousarray(packed))
            rows_d.append(
                np.ascontiguousarray(drel_flat.reshape(-1, 128).T.astype(np.float32))
            )
        idx_streams.append(rows_i)
        drel_streams.append(rows_d)

    return {
        "ntiles_qw": ntiles_qw,  # [NCHUNK, NW]
        "T_q": T_q,  # [NCHUNK]
        "idx": idx_streams,
        "drel": drel_streams,
    }


# ---------------------------------------------------------------- program

def _build_program(meta_a, meta_b):
    """meta_a: i2s relation (convs 1 and 3), meta_b: s2i relation (conv 2)."""
    import concourse.mybir as mybir
    import concourse.tile as tile
    from concourse import bacc
    from concourse.bass import ts

    FP32 = mybir.dt.float32
    BF16 = mybir.dt.bfloat16
    I16 = mybir.dt.int16
    AF = mybir.ActivationFunctionType

    nc = bacc.Bacc(
        "TRN2",
        target_bir_lowering=False,
        debug=False,
        enable_asserts=False,
        num_devices=NCORES,
        num_swdge_queues=NQUEUES,
        dynamic_dma_scratch_size=SCRATCH,
    )

    # ---- I/O
    xi0T = nc.dram_tensor("xi0T", [D, PADPER], BF16, kind="ExternalInput")
    wm1 = nc.dram_tensor("wm1", [D, HM], BF16, kind="ExternalInput")
    bm1 = nc.dram_tensor("bm1", [HM, 1], FP32, kind="ExternalInput")
    wm2b = nc.dram_tensor("wm2b", [HM + 1, D], BF16, kind="ExternalInput")
    wu1 = nc.dram_tensor("wu1", [D, HU], BF16, kind="ExternalInput")
    bu1 = nc.dram_tensor("bu1", [HU, 1], FP32, kind="ExternalInput")
    wu2 = nc.dram_tensor("wu2", [HU, D], BF16, kind="ExternalInput")
    bu2 = nc.dram_tensor("bu2", [D, 1], FP32, kind="ExternalInput")
    wob = nc.dram_tensor("wob", [D + 1, D], BF16, kind="ExternalInput")

    idx_in = {}
    drel_in = {}
    for rel, meta in (("a", meta_a), ("b", meta_b)):
        for q in range(NCHUNK):
            tqn = int(meta["T_q"][q])
            idx_in[rel, q] = nc.dram_tensor(
                f"idx_{rel}{q}", [128, tqn * 8], I16, kind="ExternalInput"
            )
            drel_in[rel, q] = nc.dram_tensor(
                f"drel_{rel}{q}", [128, tqn], FP32, kind="ExternalInput"
            )

    # output: partition-major [128, NW*64] bf16; host unpermutes
    out = nc.dram_tensor("out", [128, NW * D], BF16, kind="ExternalOutput")

    # y quarter slabs + per-(set, chunk) shared tables (sets alternate by conv)
    y_q = [nc.dram_tensor(f"y{q}", [128, QW[q] * 128], BF16) for q in range(NCHUNK)]
    tables = [
        [
            nc.dram_tensor(f"table{s}_{q}", [CHUNK_ROWS[q], 128], BF16,
                           addr_space="Shared")
            for q in range(NCHUNK)
        ]
        for s in range(2)
    ]

    iota_np = np.tile(np.arange(128, dtype=np.float32), (128, 1))
    iota_dram = nc.inline_tensor(iota_np, name="iota")
    ones_dram = nc.inline_tensor(
        np.ones((1, PADPER), ml_dtypes.bfloat16), name="onesrow"
    )

    # col tiles for the column-parallel MLPs: 24*512 + 256
    col_tiles = [(i * 512, 512) for i in range(PADPER // 512)]
    if PADPER % 512:
        col_tiles.append((PADPER - PADPER % 512, PADPER % 512))
    # col-tile index after which quarter q's y windows are complete
    q_done_ct = [6, 12, 18, len(col_tiles) - 1]

    with tile.TileContext(nc) as tc:
        with (
            tc.tile_pool(name="consts", bufs=1) as cs,
            tc.tile_pool(name="state", bufs=1) as st,
            tc.tile_pool(name="stage", bufs=3) as sg,
            tc.tile_pool(name="ipool", bufs=5) as ip,
            tc.tile_pool(name="dpool", bufs=5) as dp,
            tc.tile_pool(name="gpool", bufs=8) as gp,
            tc.tile_pool(name="spool", bufs=4) as sp,
            tc.tile_pool(name="pseg", bufs=4, space="PSUM") as pseg,
            tc.tile_pool(name="pmlp", bufs=2, space="PSUM") as pmlp,
            tc.tile_pool(name="prow", bufs=2, space="PSUM") as prow,
        ):
            # ---- constants
            iota_s = cs.tile([128, 128], FP32)
            nc.sync.dma_start(out=iota_s[:], in_=iota_dram[:, :])
            wm1_s = cs.tile([D, HM], BF16)
            nc.sync.dma_start(out=wm1_s[:], in_=wm1[:, :])
            bm1_s = cs.tile([HM, 1], FP32)
            nc.sync.dma_start(out=bm1_s[:], in_=bm1[:, :])
            wm2b_s = cs.tile([HM + 1, D], BF16)
            nc.sync.dma_start(out=wm2b_s[:], in_=wm2b[:, :])
            wu1_s = cs.tile([D, HU], BF16)
            nc.sync.dma_start(out=wu1_s[:], in_=wu1[:, :])
            bu1_s = cs.tile([HU, 1], FP32)
            nc.sync.dma_start(out=bu1_s[:], in_=bu1[:, :])
            wu2_s = cs.tile([HU, D], BF16)
            nc.sync.dma_start(out=wu2_s[:], in_=wu2[:, :])
            bu2_s = cs.tile([D, 1], FP32)
            nc.sync.dma_start(out=bu2_s[:], in_=bu2[:, :])
            wob_s = cs.tile([D + 1, D], BF16)
            nc.sync.dma_start(out=wob_s[:], in_=wob[:, :])

            # ---- persistent state
            xT = st.tile([D + 1, PADPER], BF16)  # row D = ones
            nc.sync.dma_start(out=xT[0:D, :], in_=xi0T[:, :])
            nc.sync.dma_start(out=xT[D : D + 1, :], in_=ones_dram[:, :])
            aggT = st.tile([D, PADPER], FP32)
            ybuf = st.tile([128, PADPER], BF16)  # partition-major dup'd y rows

            def fire_ag(q, tset):
                nc.gpsimd.collective_compute(
                    "AllGather",
                    mybir.AluOpType.bypass,
                    replica_groups=[list(range(NCORES))],
                    ins=[y_q[q].ap().opt()],
                    outs=[tables[tset][q].ap().opt()],
                )

            def mlp_m_ct(ct):
                """ybuf cols of col_tile ct = mlp_m(x); fire quarter y DMAs."""
                c0, cn = col_tiles[ct]
                ps = pmlp.tile([HM, 512], FP32, tag="pml")
                nc.tensor.matmul(
                    ps[:, :cn], wm1_s[:], xT[0:D, c0 : c0 + cn],
                    start=True, stop=True,
                )
                h1 = sg.tile([HM + 1, 512], BF16, tag="h1")
                nc.scalar.activation(
                    h1[0:HM, :cn], ps[:, :cn], AF.Relu, bias=bm1_s[:]
                )
                nc.vector.memset(h1[HM : HM + 1, :cn], 1.0)
                for j0 in range(0, cn, 128):
                    jj = (c0 + j0) // 128
                    ps2 = prow.tile([128, 128], FP32, tag="pro")
                    nc.tensor.matmul(
                        ps2[:, 0:D], h1[:, j0 : j0 + 128], wm2b_s[:],
                        start=True, stop=True,
                    )
                    nc.tensor.matmul(
                        ps2[:, D:128], h1[:, j0 : j0 + 128], wm2b_s[:],
                        start=True, stop=True,
                    )
                    nc.scalar.activation(
                        ybuf[:, jj * 128 : (jj + 1) * 128], ps2[:], AF.Relu
                    )
                for q in range(NCHUNK):
                    if ct == q_done_ct[q]:
                        nc.sync.dma_start(
                            out=y_q[q][:, :],
                            in_=ybuf[:, QJ0[q] * 128 : (QJ0[q] + QW[q]) * 128],
                        )

            def mlp_u_ct(ct):
                """xT cols of ct = relu(wu2.T @ relu(wu1.T @ aggT + bu1) + bu2)."""
                c0, cn = col_tiles[ct]
                ab = sg.tile([D, 512], BF16, tag="aggTb")
                nc.scalar.activation(ab[:, :cn], aggT[:, c0 : c0 + cn], AF.Copy)
                ps1 = pmlp.tile([HU, 512], FP32, tag="pml")
                nc.tensor.matmul(
                    ps1[:, :cn], wu1_s[:], ab[:, :cn], start=True, stop=True
                )
                hu = sg.tile([HU, 512], BF16, tag="hu")
                nc.scalar.activation(
                    hu[:, :cn], ps1[:, :cn], AF.Relu, bias=bu1_s[:]
                )
                ps2 = pmlp.tile([D, 512], FP32, tag="pml")
                nc.tensor.matmul(
                    ps2[:, :cn], wu2_s[:], hu[:, :cn], start=True, stop=True
                )
                nc.scalar.activation(
                    xT[0:D, c0 : c0 + cn], ps2[:, :cn], AF.Relu, bias=bu2_s[:]
                )

            def h2o_ct(ct):
                """tanh(x @ wo + bo) for ct's windows -> ybuf[:, j*64...]."""
                c0, cn = col_tiles[ct]
                for j0 in range(0, cn, 128):
                    j = (c0 + j0) // 128
                    ps = prow.tile([128, 128], FP32, tag="pro")
                    nc.tensor.matmul(
                        ps[:, 0:D], xT[:, ts(j, 128)], wob_s[:],
                        start=True, stop=True,
                    )
                    nc.scalar.activation(ybuf[:, ts(j, D)], ps[:, 0:D], AF.Tanh)

            def conv(meta, rel, tset, tail_cb=None):
                """gather + segment-sum chunk-major -> aggT."""
                ntiles_qw = meta["ntiles_qw"]
                T_q = meta["T_q"]
                callno = 0
                next_ct = 0
                ibufs, dbufs = [], []
                for q in range(NCHUNK):
                    tqn = int(T_q[q])
                    ibuf = ip.tile([128, tqn * 8], I16, tag=f"idx")
                    nc.sync.dma_start(out=ibuf[:], in_=idx_in[rel, q][:, :])
                    dbuf = dp.tile([128, tqn], FP32, tag=f"drel")
                    nc.sync.dma_start(out=dbuf[:], in_=drel_in[rel, q][:, :])
                    ibufs.append(ibuf)
                    dbufs.append(dbuf)
                for q in range(NCHUNK):
                    tqn = int(T_q[q])
                    ibuf = ibufs[q]
                    dbuf = dbufs[q]

                    calls = [(t0, min(GT, tqn - t0)) for t0 in range(0, tqn, GT)]
                    gbufs = {}
                    sbufs = {}

                    def ensure_gather(t):
                        k = t // GT
                        if k not in gbufs:
                            t0, nt = calls[k]
                            gb = gp.tile([128, GT, 128], BF16, tag="gb")
                            nc.gpsimd.dma_gather(
                                gb[:, 0:nt, :],
                                tables[tset][q][:, :],
                                ibuf[:, t0 * 8 : (t0 + nt) * 8],
                                nt * 128,
                                nt * 128,
                                128,
                                elem_step=128,
                                queue_num=(callno + k) % NQUEUES,
                                single_packet=GT <= 8,
                            )
                            gbufs[k] = gb
                        return gbufs[k], t - calls[k][0]

                    SB = 8

                    def ensure_s(t):
                        k = t // SB
                        if k not in sbufs:
                            t0 = k * SB
                            nb = min(SB, tqn - t0)
                            stile = sp.tile([128, SB, 128], BF16, tag="stile")
                            nc.vector.tensor_tensor(
                                out=stile[:, 0:nb, :],
                                in0=dbuf[:, t0 : t0 + nb].to_broadcast(
                                    [128, nb, 128]
                                ),
                                in1=iota_s[:]
                                .rearrange("p (o w) -> p o w", o=1)
                                .to_broadcast([128, nb, 128]),
                                op=mybir.AluOpType.is_equal,
                            )
                            sbufs[k] = stile
                        return sbufs[k], t - k * SB

                    tcur = 0
                    for w in range(NW):
                        nt_w = int(ntiles_qw[q, w])
                        if nt_w:
                            ps = pseg.tile([D, 128], FP32, tag="pseg")
                            for i in range(nt_w):
                                t = tcur + i
                                gb, gslot = ensure_gather(t)
                                stile, sslot = ensure_s(t)
                                nc.tensor.matmul(
                                    ps[:],
                                    gb[:, gslot, 0:D],
                                    stile[:, sslot, :],
                                    start=(i == 0),
                                    stop=(i == nt_w - 1),
                                )
                            tcur += nt_w
                            if q == 0:
                                nc.scalar.activation(
                                    aggT[:, ts(w, 128)], ps[:], AF.Copy
                                )
                            else:
                                nc.vector.tensor_tensor(
                                    out=aggT[:, ts(w, 128)],
                                    in0=ps[:],
                                    in1=aggT[:, ts(w, 128)],
                                    op=mybir.AluOpType.add,
                                )
                        if q == NCHUNK - 1 and tail_cb is not None:
                            while next_ct < len(col_tiles) and w == min(
                                4 * next_ct + 3, NW - 1
                            ):
                                tail_cb(next_ct)
                                next_ct += 1
                    callno += len(calls)

            # ---------------- the 3 convs, software-pipelined
            # conv k's chunk-3 window loop interleaves mlp_u + mlp_m (+ y DMAs
            # and quarter-AllGathers) for conv k+1 so the gather stream on the
            # GpSimd engine never drains.

            def make_tail(next_set):
                def tail(ct):
                    mlp_u_ct(ct)
                    mlp_m_ct(ct)
                    for q in range(NCHUNK - 1):
                        if ct == q_done_ct[q] + 1:
                            fire_ag(q, next_set)
                return tail

            def tail3(ct):
                mlp_u_ct(ct)
                h2o_ct(ct)

            for ct in range(len(col_tiles)):
                mlp_m_ct(ct)
                for q in range(NCHUNK):
                    if ct == q_done_ct[q]:
                        fire_ag(q, 0)

            conv(meta_a, "a", 0, tail_cb=make_tail(1))
            fire_ag(3, 1)
            conv(meta_b, "b", 1, tail_cb=make_tail(0))
            fire_ag(3, 0)
            conv(meta_a, "a", 0, tail_cb=tail3)

            # ---------------- output (staged in ybuf by h2o_ct), one DMA
            nc.sync.dma_start(out=out[:, :], in_=ybuf[:, 0 : NW * D])

    nc.compile()
    return nc


# ---------------------------------------------------------------- entry

def _prepare(
    x_served,
    x_interfered,
    edge_s2i,
    edge_i2s,
    wm1,
    bm1,
    wm2,
    bm2,
    wu1,
    bu1,
    wu2,
    bu2,
    wo,
    bo,
):
    """Host prep + program build. Returns (nc, in_maps)."""
    x_interfered = np.asarray(x_interfered, np.float32)
    e_s2i = np.asarray(edge_s2i)
    e_i2s = np.asarray(edge_i2s)

    # relation a: i2s (src interfered, dst served) -- convs 1 and 3
    meta_a = _prep_relation(e_i2s[0], e_i2s[1])
    # relation b: s2i (src served, dst interfered) -- conv 2
    meta_b = _prep_relation(e_s2i[0], e_s2i[1])

    nc = _build_program(meta_a, meta_b)

    bf = ml_dtypes.bfloat16
    wm2b = np.concatenate([wm2, bm2[None, :]], axis=0).astype(bf)
    wob = np.concatenate([wo, bo[None, :]], axis=0).astype(bf)

    in_maps = []
    for p in range(NCORES):
        xi_loc = np.zeros((D, PADPER), np.float32)
        xi_loc[:, :PERCORE] = x_interfered[p * PERCORE : (p + 1) * PERCORE].T
        m = {
            "xi0T": xi_loc.astype(bf),
            "wm1": np.ascontiguousarray(np.asarray(wm1).astype(bf)),
            "bm1": np.ascontiguousarray(np.asarray(bm1, np.float32).reshape(HM, 1)),
            "wm2b": wm2b,
            "wu1": np.ascontiguousarray(np.asarray(wu1).astype(bf)),
            "bu1": np.ascontiguousarray(np.asarray(bu1, np.float32).reshape(HU, 1)),
            "wu2": np.ascontiguousarray(np.asarray(wu2).astype(bf)),
            "bu2": np.ascontiguousarray(np.asarray(bu2, np.float32).reshape(D, 1)),
            "wob": wob,
        }
        for rel, meta in (("a", meta_a), ("b", meta_b)):
            for q in range(NCHUNK):
                m[f"idx_{rel}{q}"] = meta["idx"][p][q]
                m[f"drel_{rel}{q}"] = meta["drel"][p][q]
        in_maps.append(m)

    return nc, in_maps


def unpermute_out(raw):
    """[128, NW*D] (bf16, partition-major) -> [PERCORE, D] fp32."""
    v = np.asarray(raw).reshape(128, NW, D).transpose(1, 0, 2).reshape(PADPER, D)
    return v[:PERCORE].astype(np.float32)


def kernel(**inputs):
    from concourse.bass_utils import run_bass_kernel_spmd

    nc, in_maps = _prepare(**inputs)
    res = run_bass_kernel_spmd(
        nc, in_maps, core_ids=list(range(NCORES)), trace=TRACE
    )
    global LAST_RESULT
    LAST_RESULT = res
    outs = [unpermute_out(res.results[p]["out"]) for p in range(NCORES)]
    return np.concatenate(outs, axis=0)
